# revision 41
# baseline (speedup 1.0000x reference)
"""Trainium2 Bass kernel for nn_CustomRNN_88871463289370.

Reference computation (B=1024, T=256, H=512, HORIZON=24):
    h_0 = 0
    h_{t+1} = tanh(outer(x[:, t], Wx_w) + h_t @ Wh_w.T + (Wx_b + Wh_b))
    out = h_T @ fc_w.T + fc_b                      # [B, 24]

Strategy (data-parallel over batch, 8 cores x 128 rows each):
  * Two independent half-batch STREAMS per core (64 cols each). The
    recurrences are independent, so while stream A sits in its tanh
    latency window (PSUM -> ACT -> SBUF -> sem, ~840 ns), the PE runs
    stream B's matmuls and vice versa. The per-step critical cycle is
    one stream's ACT round-trip (~640 ns) + its 8 recurrent matmuls
    (~105 ns) + the PSUM->ACT hop (~200 ns) = 941 ns/step. More than
    2 streams loses: each extra ACT instruction costs a fixed ~370 ns
    access-latency bubble, making 3+ streams ACT-throughput-bound.
  * Recurrent matmuls in fp8e4m3 with DoubleRow perf mode: one MM
    contracts TWO 128-row k-chunks (weights [128, 2, 128], moving
    h [128, 2, 64]) at 0.5 cycles/row -- 4x less PE time than bf16.
  * Precision: Wh is pre-scaled by 16 before the fp8 cast (a third of
    the uniform(+-1/sqrt(512)) weights sit below e4m3's min normal
    2^-6 unscaled); the tanh activation applies scale=1/16 to PSUM.
    The x-outer phase and its bias stay bf16 (also pre-scaled by 16,
    exact since 16 is a power of two). fp8 h (+-1, ~2% rms quant
    error) leaves a steady-state recurrence error ~1.3e-2, but the
    recurrence is contracting (spectral radius ~0.58), so the LAST
    `TAIL` steps run in bf16 (plain K=128 matmuls), collapsing the
    final error to ~5e-3 (gate: 2e-2).
  * Per stream-step: one [128, 4, 64] PSUM tile (4 hidden m-chunks x
    64 batch cols, 1 KiB -- its own bank), 12 accumulating MMs
    (4 bf16 x-outer K=2 + 8 fp8-DR), one [128, 256] Tanh ACT that
    also casts to the dtype the next step's MMs need (fp8 or bf16).
  * All inputs SBUF-resident after the up-front DMAs; no per-step DMA.
  * Final projection in bf16 from the last step's bf16 h tiles:
    8 K=128 N=64 matmuls into a [24, 128] PSUM tile + Identity ACT
    with per-partition bias.

All host-side reshaping/transposition/casting happens in kernel();
the device kernel sees pre-massaged tensors.
"""

import numpy as np
import ml_dtypes

HIDDEN = 512
HORIZON = 24
B_FULL = 1024
T_FULL = 256
N_CORES = 8
B_CORE = B_FULL // N_CORES  # 128
KC = HIDDEN // 128          # 4 chunks of the hidden dim
NS = 2                      # independent half-batch streams per core
BS = B_CORE // NS           # 64 batch cols per stream
TAIL = 2                    # trailing steps run in bf16 to wash out fp8 error
                            # (numpy-exact sim: rel err 0.0053 vs 2e-2 gate;
                            # HW has tracked the sim within ~1% relative)
N_BOOT_X = 6                # x columns carried in the boot DMA (bf16 ph0);
                            # from t=N_BOOT_X the fp8 x8 tensor has landed
WH_SCALE = 16.0             # power of two: exact to undo via ACT scale

_COMPILED = {}


# _relax_tanh_waits is kept for documentation but DISABLED: both variants
# (reducing ACT wait thresholds out-of-band; moving trailing MM sem-incs
# onto an earlier anchor MM) break downstream layers -- walrus re-derives
# its physical-semaphore schedule from the emitted counts and the edited
# program wedges the device (NRT INTERNAL on both HW attempts, baseline
# healthy in between), and TimelineSim hardcodes sem-inc as +1 so the
# anchor variant deadlocks in sim.  The ~52 ns/step it would save is not
# expressible through Tile's data-dependency model.
RELAX = False


def build_kernel(T=T_FULL, use_bf16=True, tail=TAIL):
    import concourse.bass as bass
    import concourse.mybir as mybir
    import concourse.tile as tile
    from concourse.bass import ts

    dtb = mybir.dt.bfloat16
    dt8 = mybir.dt.float8e4
    f32 = mybir.dt.float32
    DR = mybir.MatmulPerfMode.DoubleRow

    nc = bass.Bass("TRN2", target_bir_lowering=False, debug=False,
                   num_devices=N_CORES)

    # ---- DRAM I/O (per-core shapes; host pre-massages layouts) ----
    # x2T[0, t] = x[:, t], x2T[1, t] = ones; shape [2, T, B_CORE]
    x2T_d = nc.dram_tensor("x2T", [2, T, B_CORE], dtb, kind="ExternalInput").ap()
    # boot[:, :H] = [16*Wx_w ; 16*(Wx_b + Wh_b)], boot[:, H + t*B : H + (t+1)*B]
    # = [x[:, t] ; ones] for t < N_BOOT_X
    boot_d = nc.dram_tensor("boot", [2, HIDDEN + N_BOOT_X * B_CORE], dtb,
                            kind="ExternalInput").ap()
    # fp8 x-outer operands for steps N_BOOT_X..n_fp8-1, one tensor:
    # x8[:, :H] = [16*Wx_w ; 16*(Wx_b+Wh_b)], x8[:, H + t*B + b] = [x[b,t] ; 1]
    # (keeps each fp8 step's PSUM accumulation group single-dtype -- the
    # mixed bf16+fp8 groups are the prime suspect for a rare
    # NRT_EXEC_UNIT_UNRECOVERABLE wedge seen once in ~9 HW runs)
    x8_d = nc.dram_tensor("x8", [2, HIDDEN + T * B_CORE], dt8,
                          kind="ExternalInput").ap()
    # bf16 tail weights [128, KC, H]: whT[p, k, m] = 16*Wh_w[m, k*128+p]
    whT_d = nc.dram_tensor("whT", [128, KC, HIDDEN], dtb, kind="ExternalInput").ap()
    # fp8 DoubleRow weights [128, 2, 2, H]:
    #   wh8[p, pr, i, m] = 16*Wh_w[m, (2*pr+i)*128+p]
    wh8_d = nc.dram_tensor("wh8", [128, 2, 2, HIDDEN], dt8, kind="ExternalInput").ap()
    # fcT arranged [128, KC, HORIZON]: fcT[p, k, n] = fc_w[n, k*128+p] (unscaled)
    fcT_d = nc.dram_tensor("fcT", [128, KC, HORIZON], dtb, kind="ExternalInput").ap()
    # fc_b as column [HORIZON, 1] fp32
    fcb_d = nc.dram_tensor("fcb", [HORIZON, 1], f32, kind="ExternalInput").ap()
    # output [HORIZON, B_CORE] fp32 (host transposes/concats)
    out_d = nc.dram_tensor("out", [HORIZON, B_CORE], f32, kind="ExternalOutput").ap()

    n_fp8 = max(0, T - tail)  # steps 1..n_fp8-1 use fp8-DR recurrent MMs

    with tile.TileContext(nc) as tc:
        with (
            tc.tile_pool(name="consts", bufs=1) as cpool,
            tc.tile_pool(name="h", bufs=3) as hpool,
            tc.tile_pool(name="ps", bufs=3, space="PSUM") as pspool,
            tc.tile_pool(name="psf", bufs=1, space="PSUM") as psfpool,
            tc.tile_pool(name="fin", bufs=1) as finpool,
        ):
            # ---- load constants into SBUF ----
            # Issue order matters: the SP engine serializes dma_start issues
            # (~650 ns each). t=0 needs wxb+x2, t=1 needs wh8; whT is only
            # read from step T-TAIL (~230 us in), so it goes last.
            # "boot" tile: wxb + the first N_BOOT_X x columns in ONE first
            # DMA -- the serial per-DMA issue+descgen+completion latency
            # (~2 us each) is what gates the first matmuls, so everything
            # steps 0..N_BOOT_X-1 need (except wh8) rides together.  wh8
            # goes second (it gates step 1's recurrent MMs); the bulk x
            # transfer third (first read at t=N_BOOT_X, ~1.5 us later).
            boot_sb = cpool.tile([2, HIDDEN + N_BOOT_X * B_CORE], dtb)
            nc.sync.dma_start(boot_sb[:], boot_d[:])
            wh8_sb = cpool.tile([128, 2, 2, HIDDEN], dt8)
            nc.sync.dma_start(wh8_sb[:], wh8_d[:])
            x8_sb = cpool.tile([2, HIDDEN + T * B_CORE], dt8)
            nc.sync.dma_start(x8_sb[:], x8_d[:])
            fcT_sb = cpool.tile([128, KC, HORIZON], dtb)
            nc.sync.dma_start(fcT_sb[:], fcT_d[:])
            fcb_sb = cpool.tile([HORIZON, 1], f32)
            nc.sync.dma_start(fcb_sb[:], fcb_d[:])
            # x2 (bf16) is only read by the bf16 TAIL steps (~230 us in);
            # whT likewise -- both ride at the back of the queue.
            x2_sb = cpool.tile([2, T, B_CORE], dtb)
            nc.sync.dma_start(x2_sb[:], x2T_d[:])
            whT_sb = cpool.tile([128, KC, HIDDEN], dtb)
            nc.sync.dma_start(whT_sb[:], whT_d[:])
            # Touch fcb on ScalarE right away so the DMA wait lands here,
            # not on the final bias activation (which already carries a PE
            # wait; the AC instruction struct fits only one sync wait).
            fcb_scratch = cpool.tile([1, 1], f32)
            nc.scalar.activation(fcb_scratch[:], fcb_sb[0:1, 0:1],
                                 mybir.ActivationFunctionType.Identity)

            inv = 1.0 / WH_SCALE
            h = [None, None]  # per-stream [128, KC, BS] tiles

            for t in range(T):
                fp8_mm = 0 < t < n_fp8       # this step's recurrent MM flavor
                fp8_out = (t + 1) < n_fp8    # dtype the NEXT step's MMs need
                for s in range(NS):
                    ps = pspool.tile([128, KC, BS], f32, tag=f"ps{s}")
                    # x-outer + bias (K=2). fp8 during the fp8 steps so the
                    # whole accumulation group is one dtype; bf16 (boot/x2)
                    # for t<N_BOOT_X and the tail. One start per PSUM bank,
                    # one stop on the bank's last accumulating MM.
                    if N_BOOT_X <= t < n_fp8:
                        off = HIDDEN + t * B_CORE + s * BS
                        wsrc, xsrc = x8_sb, x8_sb[0:2, off:off + BS]
                    else:
                        wsrc = boot_sb
                        xsrc = (boot_sb[0:2, HIDDEN + t * B_CORE + s * BS:
                                        HIDDEN + t * B_CORE + (s + 1) * BS]
                                if t < N_BOOT_X else x2_sb[0:2, t, ts(s, BS)])
                    for m in range(KC):
                        nc.tensor.matmul(ps[:, m, :],
                                         wsrc[0:2, ts(m, 128)],
                                         xsrc,
                                         start=(m == 0),
                                         stop=(t == 0 and m == KC - 1))
                    if t > 0:
                        if fp8_mm:
                            # fp8 DoubleRow: contract k-chunk pair (2pr, 2pr+1)
                            for m in range(KC):
                                for pr in range(2):
                                    nc.tensor.matmul(
                                        ps[:, m, :],
                                        wh8_sb[:, pr, :, ts(m, 128)],
                                        h[s][:, 2 * pr:2 * pr + 2, :],
                                        start=False,
                                        stop=(m == KC - 1 and pr == 1),
                                        perf_mode=DR)
                        else:
                            for m in range(KC):
                                for k in range(KC):
                                    nc.tensor.matmul(
                                        ps[:, m, :],
                                        whT_sb[:, k, ts(m, 128)],
                                        h[s][:, k, :],
                                        start=False,
                                        stop=(m == KC - 1 and k == KC - 1))
                    htag = f"h8{s}" if fp8_out else f"hb{s}"
                    h_new = hpool.tile([128, KC, BS], dt8 if fp8_out else dtb,
                                       tag=htag)
                    nc.scalar.activation(h_new[:], ps[:],
                                         mybir.ActivationFunctionType.Tanh,
                                         scale=inv)
                    h[s] = h_new

            # ---- final projection: out[n, b] = sum_k fcT[k].T @ h[k] + b ----
            # Per stream so stream A's bias-ACT + store overlap stream B's
            # last tanh and fc matmuls.
            for s in range(NS):
                ps_fc = psfpool.tile([HORIZON, BS], f32, tag=f"psfc{s}")
                for k in range(KC):
                    nc.tensor.matmul(ps_fc[:],
                                     fcT_sb[:, k, :],
                                     h[s][:, k, :],
                                     start=(k == 0),
                                     stop=(k == KC - 1))
                out_sb = finpool.tile([HORIZON, BS], f32, tag=f"out{s}")
                nc.scalar.activation(out_sb[:], ps_fc[:],
                                     mybir.ActivationFunctionType.Identity,
                                     bias=fcb_sb[:])
                nc.sync.dma_start(out_d[:, ts(s, BS)], out_sb[:])

    _strip_redundant_self_waits(nc)
    if RELAX:
        _relax_tanh_waits(nc, mybir, n_fp8)
    return nc


def _relax_tanh_waits(nc, mybir, n_fp8):
    """Point each steady-state Tanh ACT's PE-sem wait a few matmuls EARLIER
    than the last accumulating MM of its PSUM tile.

    The PSUM->ACT handoff costs ~199 ns (SEM_DELAY 100 + seq fetch/decode +
    dispatch) measured from the sem update of the MM the ACT waits on.  The
    trailing DR MMs of the burst only need 13 ns each (27 ns for the bf16
    tail), so waiting on MM #N-delta keeps the data-complete point well
    inside the handoff latency while starting the handoff earlier --
    removing delta MM productions from the serial recurrence cycle.

    Margins (sim-calibrated): fp8 steps delta=6 -> 199-6*13 = 121 ns; bf16
    tail delta=2 -> 199-2*27 = 145 ns.  The 100 ns hardware semaphore
    propagation alone covers the trailing work in both cases (78 ns / 54 ns
    of trailing MM production).  Steps t<2 are skipped (their MMs can run
    at low p-state, 2-4x slower).

    Mechanism: wait VALUES are left untouched (walrus re-derives its
    physical-sem schedule from them; editing a threshold out-of-band
    wedges the device).  Instead the trailing delta MMs' sem-inc updates
    are MOVED onto the (delta+1)-th-from-last MM (update_value 1+delta),
    so the existing threshold is reached delta MMs earlier.  Sem totals
    are unchanged for every later waiter; a pre-pass asserts no OTHER
    instruction waits inside the moved window.
    """
    instrs = [i for b in nc.m.functions[0].blocks for i in b.instructions]

    # Program-order PE Matmults with cumulative per-sem counts, and every
    # wait in the module keyed by sem.
    mms = []                 # (inst, sem, cum_after)
    cum = {}
    all_waits = {}           # sem -> sorted list of (value, inst)
    for i in instrs:
        si = i.sync_info
        if si is None:
            continue
        if type(i).__name__ == "InstMatmult":
            for u in si.on_update:
                assert u.update_mode == "sem-inc" and (u.update_value or 1) == 1
                cum[u.ant_name] = cum.get(u.ant_name, 0) + 1
                mms.append((i, u.ant_name, cum[u.ant_name]))
        for w in si.on_wait:
            if w.wait_mode == "sem-ge-imm":
                all_waits.setdefault(w.ant_name, []).append((w.wait_value, i))

    by_sem_cum = {}          # (sem, cum) -> mm index in mms
    for j, (i, sem, c) in enumerate(mms):
        by_sem_cum[(sem, c)] = j

    t_s = [(t, s) for t in range(T_FULL) for s in range(NS)]
    n_tanh = 0
    for i in instrs:
        if (type(i).__name__ != "InstActivation"
                or i.func != mybir.ActivationFunctionType.Tanh):
            continue
        t, _s = t_s[n_tanh]
        n_tanh += 1
        if t < 2:
            continue
        only = getattr(_relax_tanh_waits, "_only", None)
        if only is not None and (t, _s) not in only:
            continue
        delta = 6 if t < n_fp8 else 2
        si = i.sync_info
        pe_waits = [w for w in (si.on_wait if si else [])
                    if w.wait_mode == "sem-ge-imm" and (w.ant_name, w.wait_value) in by_sem_cum]
        if len(pe_waits) != 1:
            continue
        w = pe_waits[0]
        sem, v = w.ant_name, w.wait_value
        # Nothing else may wait inside (v-delta, v].
        others = [wi for (val, wi) in all_waits.get(sem, ())
                  if v - delta < val <= v and wi is not i]
        if others:
            continue
        j_last = by_sem_cum[(sem, v)]
        j_new = by_sem_cum.get((sem, v - delta))
        if j_new is None or j_last - j_new != delta:
            continue  # window not a contiguous MM run; leave as-is
        # Move the trailing delta increments onto MM j_new.  Attribute
        # mutation on a SyncUpdate does not persist (pyo3 copies), so the
        # on_update LIST is rebuilt with a fresh object.
        import bass_rust
        for j in range(j_new + 1, j_last + 1):
            mi = mms[j][0]
            mi.sync_info.on_update = [u for u in mi.sync_info.on_update
                                      if u.ant_name != sem]
        anchor = mms[j_new][0]
        new_ups = []
        for u in anchor.sync_info.on_update:
            if u.ant_name == sem:
                u = bass_rust.SyncUpdate(
                    sync_type=u.sync_type, id=u.id, update_mode=u.update_mode,
                    ant_name=u.ant_name, update_value=1 + delta,
                    update_reg=u.update_reg)
            new_ups.append(u)
        anchor.sync_info.on_update = new_ups


_SELF_SEM_PREFIX = {
    "InstActivation": "Activation",
    "InstMatmult": "PE",
    "InstLdweights": "PE",
    "InstTensorTensor": "DVE",
    "InstTensorScalarPtr": "DVE",
    "InstTensorCopy": "DVE",
}


def _strip_redundant_self_waits(nc):
    """Drop same-engine semaphore waits from instructions that carry more
    than one sync wait.

    Rationale: the HW engine instruction structs (MM/AC) hold only ONE
    sync-wait command; walrus refuses to codegen instructions with two.
    Tile emits a wait on the instruction's own engine sem for WAW/WAR on
    recycled tile-pool slots, but each engine executes its queue strictly
    in order, so ordering vs. its own earlier instructions is guaranteed
    without the wait.  Cross-engine waits are preserved; sem update counts
    are untouched (no other wait thresholds shift).
    """
    # Semaphore updated by the final DMA store of the "out" tensor; the
    # kernel-tail drain only genuinely needs this one (everything else is
    # transitively ordered: input DMAs -> compute -> final ACT -> out DMA).
    out_dma_sems = set()
    for b in nc.m.functions[0].blocks:
        for i in b.instructions:
            if type(i).__name__ != "InstDMACopy":
                continue
            names = [getattr(ap, "memref", "") for ap in i.outs]
            if "out" in names:
                si = i.sync_info
                if si:
                    out_dma_sems.update(u.ant_name for u in si.on_update)

    for b in nc.m.functions[0].blocks:
        for i in b.instructions:
            si = i.sync_info
            if si is None:
                continue
            ow = si.on_wait
            if len(ow) < 2:
                continue
            tname = type(i).__name__
            if tname == "InstDrain" and any(
                w.ant_name in out_dma_sems for w in ow
            ):
                si.on_wait = [w for w in ow if w.ant_name in out_dma_sems][:1]
                continue
            if tname == "InstDMACopy":
                # Keep the compute-engine wait (real data dependency);
                # drop stale cross-queue DMAHW waits (no data dependency:
                # all earlier DMAs here are input preloads this store
                # does not read, and same-ring descriptors are ordered
                # by the ring itself).
                kept = [w for w in ow if not w.ant_name.startswith("DMA")]
                if kept and len(kept) < len(ow):
                    si.on_wait = kept
                continue
            self_prefix = _SELF_SEM_PREFIX.get(tname)
            if self_prefix is None:
                continue
            kept = [w for w in ow if not w.ant_name.startswith(self_prefix)]
            if kept and len(kept) < len(ow):
                si.on_wait = kept


def _prep_inputs(x, Wx_w, Wx_b, Wh_w, Wh_b, fc_w, fc_b, T):
    """Host-side shard + layout massaging. Returns per-core input maps."""
    bf = ml_dtypes.bfloat16
    f8 = ml_dtypes.float8_e4m3
    bias = (Wx_b + Wh_b).astype(np.float32)

    wxb = WH_SCALE * np.stack([Wx_w.astype(np.float32), bias])       # [2, H]
    whT16 = WH_SCALE * Wh_w.T.astype(np.float32)                     # [Hk, Hm]
    whT = whT16.reshape(KC, 128, HIDDEN).transpose(1, 0, 2).copy().astype(bf)
    wh8 = whT16.reshape(2, 2, 128, HIDDEN).transpose(2, 0, 1, 3).copy().astype(f8)
    fcT = (fc_w.T.astype(np.float32)
           .reshape(KC, 128, HORIZON).transpose(1, 0, 2).copy().astype(bf))
    fcb = fc_b.astype(np.float32).reshape(HORIZON, 1).copy()

    in_maps = []
    for c in range(N_CORES):
        xs = x[c * B_CORE:(c + 1) * B_CORE, :T]                      # [128, T]
        x2T = np.empty((2, T, B_CORE), dtype=np.float32)
        x2T[0] = xs.T
        x2T[1] = 1.0
        boot = np.concatenate(
            [wxb] + [x2T[:, u, :] for u in range(N_BOOT_X)], axis=1)
        x8 = np.concatenate([wxb, x2T.reshape(2, T * B_CORE)], axis=1)
        in_maps.append({
            "x2T": x2T.astype(bf),
            "boot": boot.astype(bf),
            "x8": x8.astype(f8),
            "whT": whT,
            "wh8": wh8,
            "fcT": fcT,
            "fcb": fcb,
        })
    return in_maps


def kernel(x, Wx_w, Wx_b, Wh_w, Wh_b, fc_w, fc_b, _T=T_FULL, _bf16=True,
           _trace=False):
    from concourse.bass_utils import run_bass_kernel_spmd

    key = (_T, _bf16)
    if key not in _COMPILED:
        _COMPILED[key] = build_kernel(T=_T)
    nc = _COMPILED[key]

    in_maps = _prep_inputs(x, Wx_w, Wx_b, Wh_w, Wh_b, fc_w, fc_b, _T)
    try:
        res = run_bass_kernel_spmd(nc, in_maps, list(range(N_CORES)),
                                   trace=_trace)
    except Exception:
        # A rare transient NRT_EXEC_UNIT_UNRECOVERABLE wedge has been seen
        # (~1 in 9 runs); the device recovers on re-execution.
        res = run_bass_kernel_spmd(nc, in_maps, list(range(N_CORES)),
                                   trace=_trace)
    outs = [res.results[c]["out"] for c in range(N_CORES)]               # [24, 128] each
    full = np.concatenate(outs, axis=1).T.astype(np.float32).copy()      # [1024, 24]
    kernel._last_result = res
    return full


# revision 43
# speedup vs baseline: 1.0003x; 1.0003x over previous
"""Trainium2 Bass kernel for nn_CustomRNN_88871463289370.

Reference computation (B=1024, T=256, H=512, HORIZON=24):
    h_0 = 0
    h_{t+1} = tanh(outer(x[:, t], Wx_w) + h_t @ Wh_w.T + (Wx_b + Wh_b))
    out = h_T @ fc_w.T + fc_b                      # [B, 24]

Strategy (data-parallel over batch, 8 cores x 128 rows each):
  * Two independent half-batch STREAMS per core (64 cols each). The
    recurrences are independent, so while stream A sits in its tanh
    latency window (PSUM -> ACT -> SBUF -> sem, ~840 ns), the PE runs
    stream B's matmuls and vice versa. The per-step critical cycle is
    one stream's ACT round-trip (~640 ns) + its 8 recurrent matmuls
    (~105 ns) + the PSUM->ACT hop (~200 ns) = 941 ns/step. More than
    2 streams loses: each extra ACT instruction costs a fixed ~370 ns
    access-latency bubble, making 3+ streams ACT-throughput-bound.
  * Recurrent matmuls in fp8e4m3 with DoubleRow perf mode: one MM
    contracts TWO 128-row k-chunks (weights [128, 2, 128], moving
    h [128, 2, 64]) at 0.5 cycles/row -- 4x less PE time than bf16.
  * Precision: Wh is pre-scaled by 16 before the fp8 cast (a third of
    the uniform(+-1/sqrt(512)) weights sit below e4m3's min normal
    2^-6 unscaled); the tanh activation applies scale=1/16 to PSUM.
    The x-outer phase and its bias stay bf16 (also pre-scaled by 16,
    exact since 16 is a power of two). fp8 h (+-1, ~2% rms quant
    error) leaves a steady-state recurrence error ~1.3e-2, but the
    recurrence is contracting (spectral radius ~0.58), so the LAST
    `TAIL` steps run in bf16 (plain K=128 matmuls), collapsing the
    final error to ~5e-3 (gate: 2e-2).
  * Per stream-step: one [128, 4, 64] PSUM tile (4 hidden m-chunks x
    64 batch cols, 1 KiB -- its own bank), 12 accumulating MMs
    (4 bf16 x-outer K=2 + 8 fp8-DR), one [128, 256] Tanh ACT that
    also casts to the dtype the next step's MMs need (fp8 or bf16).
  * All inputs SBUF-resident after the up-front DMAs; no per-step DMA.
  * Final projection in bf16 from the last step's bf16 h tiles:
    8 K=128 N=64 matmuls into a [24, 128] PSUM tile + Identity ACT
    with per-partition bias.

All host-side reshaping/transposition/casting happens in kernel();
the device kernel sees pre-massaged tensors.
"""

import numpy as np
import ml_dtypes

HIDDEN = 512
HORIZON = 24
B_FULL = 1024
T_FULL = 256
N_CORES = 8
B_CORE = B_FULL // N_CORES  # 128
KC = HIDDEN // 128          # 4 chunks of the hidden dim
NS = 2                      # independent half-batch streams per core
BS = B_CORE // NS           # 64 batch cols per stream
TAIL = 2                    # trailing steps run in bf16 to wash out fp8 error
                            # (numpy-exact sim: rel err 0.0053 vs 2e-2 gate;
                            # HW has tracked the sim within ~1% relative)
N_BOOT_X = 6                # x columns carried in the boot DMA (bf16 ph0);
                            # from t=N_BOOT_X the fp8 x8 tensor has landed
WH_SCALE = 16.0             # power of two: exact to undo via ACT scale

_COMPILED = {}


# _relax_tanh_waits is kept for documentation but DISABLED: both variants
# (reducing ACT wait thresholds out-of-band; moving trailing MM sem-incs
# onto an earlier anchor MM) break downstream layers -- walrus re-derives
# its physical-semaphore schedule from the emitted counts and the edited
# program wedges the device (NRT INTERNAL on both HW attempts, baseline
# healthy in between), and TimelineSim hardcodes sem-inc as +1 so the
# anchor variant deadlocks in sim.  The ~52 ns/step it would save is not
# expressible through Tile's data-dependency model.
RELAX = False


def build_kernel(T=T_FULL, use_bf16=True, tail=TAIL):
    import concourse.bass as bass
    import concourse.mybir as mybir
    import concourse.tile as tile
    from concourse.bass import ts

    dtb = mybir.dt.bfloat16
    dt8 = mybir.dt.float8e4
    f32 = mybir.dt.float32
    DR = mybir.MatmulPerfMode.DoubleRow

    nc = bass.Bass("TRN2", target_bir_lowering=False, debug=False,
                   num_devices=N_CORES)

    # ---- DRAM I/O (per-core shapes; host pre-massages layouts) ----
    # x2T[0, t] = x[:, t], x2T[1, t] = ones; shape [2, T, B_CORE]
    x2T_d = nc.dram_tensor("x2T", [2, T, B_CORE], dtb, kind="ExternalInput").ap()
    # boot[:, :H] = [16*Wx_w ; 16*(Wx_b + Wh_b)], boot[:, H + t*B : H + (t+1)*B]
    # = [x[:, t] ; ones] for t < N_BOOT_X
    boot_d = nc.dram_tensor("boot", [2, HIDDEN + N_BOOT_X * B_CORE], dtb,
                            kind="ExternalInput").ap()
    # fp8 x-outer operands for steps N_BOOT_X..n_fp8-1, one tensor:
    # x8[:, :H] = [16*Wx_w ; 16*(Wx_b+Wh_b)], x8[:, H + t*B + b] = [x[b,t] ; 1]
    # (keeps each fp8 step's PSUM accumulation group single-dtype -- the
    # mixed bf16+fp8 groups are the prime suspect for a rare
    # NRT_EXEC_UNIT_UNRECOVERABLE wedge seen once in ~9 HW runs)
    x8_d = nc.dram_tensor("x8", [2, HIDDEN + T * B_CORE], dt8,
                          kind="ExternalInput").ap()
    # bf16 tail weights [128, KC, H]: whT[p, k, m] = 16*Wh_w[m, k*128+p]
    whT_d = nc.dram_tensor("whT", [128, KC, HIDDEN], dtb, kind="ExternalInput").ap()
    # fp8 DoubleRow weights [128, 2, 2, H]:
    #   wh8[p, pr, i, m] = 16*Wh_w[m, (2*pr+i)*128+p]
    wh8_d = nc.dram_tensor("wh8", [128, 2, 2, HIDDEN], dt8, kind="ExternalInput").ap()
    # fcT arranged [128, KC, HORIZON]: fcT[p, k, n] = fc_w[n, k*128+p] (unscaled)
    fcT_d = nc.dram_tensor("fcT", [128, KC, HORIZON], dtb, kind="ExternalInput").ap()
    # fc_b as column [HORIZON, 1] fp32
    fcb_d = nc.dram_tensor("fcb", [HORIZON, 1], f32, kind="ExternalInput").ap()
    # output [HORIZON, B_CORE] fp32 (host transposes/concats)
    out_d = nc.dram_tensor("out", [HORIZON, B_CORE], f32, kind="ExternalOutput").ap()

    n_fp8 = max(0, T - tail)  # steps 1..n_fp8-1 use fp8-DR recurrent MMs

    with tile.TileContext(nc) as tc:
        with (
            tc.tile_pool(name="consts", bufs=1) as cpool,
            tc.tile_pool(name="h", bufs=3) as hpool,
            tc.tile_pool(name="ps", bufs=3, space="PSUM") as pspool,
            tc.tile_pool(name="psf", bufs=1, space="PSUM") as psfpool,
            tc.tile_pool(name="fin", bufs=1) as finpool,
        ):
            # ---- load constants into SBUF ----
            # Issue order matters: the SP engine serializes dma_start issues
            # (~650 ns each). t=0 needs wxb+x2, t=1 needs wh8; whT is only
            # read from step T-TAIL (~230 us in), so it goes last.
            # "boot" tile: wxb + the first N_BOOT_X x columns in ONE first
            # DMA -- the serial per-DMA issue+descgen+completion latency
            # (~2 us each) is what gates the first matmuls, so everything
            # steps 0..N_BOOT_X-1 need (except wh8) rides together.  wh8
            # goes second (it gates step 1's recurrent MMs); the bulk x
            # transfer third (first read at t=N_BOOT_X, ~1.5 us later).
            boot_sb = cpool.tile([2, HIDDEN + N_BOOT_X * B_CORE], dtb)
            nc.sync.dma_start(boot_sb[:], boot_d[:])
            wh8_sb = cpool.tile([128, 2, 2, HIDDEN], dt8)
            nc.sync.dma_start(wh8_sb[:], wh8_d[:])
            x8_sb = cpool.tile([2, HIDDEN + T * B_CORE], dt8)
            nc.sync.dma_start(x8_sb[:], x8_d[:])
            fcT_sb = cpool.tile([128, KC, HORIZON], dtb)
            nc.sync.dma_start(fcT_sb[:], fcT_d[:])
            fcb_sb = cpool.tile([HORIZON, 1], f32)
            nc.sync.dma_start(fcb_sb[:], fcb_d[:])
            # x2 (bf16) is only read by the bf16 TAIL steps (~230 us in);
            # whT likewise -- both ride at the back of the queue.
            x2_sb = cpool.tile([2, T, B_CORE], dtb)
            nc.sync.dma_start(x2_sb[:], x2T_d[:])
            whT_sb = cpool.tile([128, KC, HIDDEN], dtb)
            nc.sync.dma_start(whT_sb[:], whT_d[:])
            # Touch fcb on ScalarE right away so the DMA wait lands here,
            # not on the final bias activation (which already carries a PE
            # wait; the AC instruction struct fits only one sync wait).
            fcb_scratch = cpool.tile([1, 1], f32)
            nc.scalar.activation(fcb_scratch[:], fcb_sb[0:1, 0:1],
                                 mybir.ActivationFunctionType.Identity)

            inv = 1.0 / WH_SCALE
            h = [None, None]  # per-stream [128, KC, BS] tiles

            def hk(hs, k):
                """Chunk k of a previous-step h: single tile or (h01, h23)."""
                if isinstance(hs, tuple):
                    return hs[k // 2][:, k % 2, :]
                return hs[:, k, :]

            for t in range(T):
                fp8_mm = 0 < t < n_fp8       # this step's recurrent MM flavor
                fp8_out = (t + 1) < n_fp8    # dtype the NEXT step's MMs need
                if t >= n_fp8:
                    # bf16 TAIL step, k-phase split: two half-PSUMs + two
                    # half-ACTs per stream so the m01 tanh (and with it the
                    # next step's k01 matmuls / the fc projection) starts
                    # ~200 ns earlier.  Summation order per element is
                    # unchanged (ph0 then k=0..3), so results are
                    # bit-identical to the unsplit path.
                    for s in range(NS):
                        halves = []
                        for hh in range(2):          # hh=0: m0,m1; 1: m2,m3
                            psh = pspool.tile([128, 2, BS], f32, tag=f"ps{hh}")
                            xsrc = x2_sb[0:2, t, ts(s, BS)]
                            for mm in range(2):
                                m = 2 * hh + mm
                                nc.tensor.matmul(psh[:, mm, :],
                                                 boot_sb[0:2, ts(m, 128)],
                                                 xsrc,
                                                 start=(mm == 0), stop=False)
                            for mm in range(2):
                                m = 2 * hh + mm
                                for k in range(KC):
                                    nc.tensor.matmul(
                                        psh[:, mm, :],
                                        whT_sb[:, k, ts(m, 128)],
                                        hk(h[s], k),
                                        start=False,
                                        stop=(mm == 1 and k == KC - 1))
                            h_half = hpool.tile([128, 2, BS], dtb,
                                                tag=f"hb{s}")
                            nc.scalar.activation(
                                h_half[:], psh[:],
                                mybir.ActivationFunctionType.Tanh, scale=inv)
                            halves.append(h_half)
                        h[s] = (halves[0], halves[1])
                    continue
                for s in range(NS):
                    ps = pspool.tile([128, KC, BS], f32, tag=f"ps{s}")
                    # x-outer + bias (K=2). fp8 during the fp8 steps so the
                    # whole accumulation group is one dtype; bf16 (boot/x2)
                    # for t<N_BOOT_X and the tail. One start per PSUM bank,
                    # one stop on the bank's last accumulating MM.
                    if N_BOOT_X <= t < n_fp8:
                        off = HIDDEN + t * B_CORE + s * BS
                        wsrc, xsrc = x8_sb, x8_sb[0:2, off:off + BS]
                    else:
                        wsrc = boot_sb
                        xsrc = (boot_sb[0:2, HIDDEN + t * B_CORE + s * BS:
                                        HIDDEN + t * B_CORE + (s + 1) * BS]
                                if t < N_BOOT_X else x2_sb[0:2, t, ts(s, BS)])
                    for m in range(KC):
                        nc.tensor.matmul(ps[:, m, :],
                                         wsrc[0:2, ts(m, 128)],
                                         xsrc,
                                         start=(m == 0),
                                         stop=(t == 0 and m == KC - 1))
                    if t > 0:
                        if fp8_mm:
                            # fp8 DoubleRow: contract k-chunk pair (2pr, 2pr+1)
                            for m in range(KC):
                                for pr in range(2):
                                    nc.tensor.matmul(
                                        ps[:, m, :],
                                        wh8_sb[:, pr, :, ts(m, 128)],
                                        h[s][:, 2 * pr:2 * pr + 2, :],
                                        start=False,
                                        stop=(m == KC - 1 and pr == 1),
                                        perf_mode=DR)
                        else:
                            for m in range(KC):
                                for k in range(KC):
                                    nc.tensor.matmul(
                                        ps[:, m, :],
                                        whT_sb[:, k, ts(m, 128)],
                                        h[s][:, k, :],
                                        start=False,
                                        stop=(m == KC - 1 and k == KC - 1))
                    htag = f"h8{s}" if fp8_out else f"hb{s}"
                    h_new = hpool.tile([128, KC, BS], dt8 if fp8_out else dtb,
                                       tag=htag)
                    nc.scalar.activation(h_new[:], ps[:],
                                         mybir.ActivationFunctionType.Tanh,
                                         scale=inv)
                    h[s] = h_new

            # ---- final projection: out[n, b] = sum_k fcT[k].T @ h[k] + b ----
            # Per stream so stream A's bias-ACT + store overlap stream B's
            # last tanh and fc matmuls.
            for s in range(NS):
                ps_fc = psfpool.tile([HORIZON, BS], f32, tag=f"psfc{s}")
                for k in range(KC):
                    nc.tensor.matmul(ps_fc[:],
                                     fcT_sb[:, k, :],
                                     hk(h[s], k),
                                     start=(k == 0),
                                     stop=(k == KC - 1))
                out_sb = finpool.tile([HORIZON, BS], f32, tag=f"out{s}")
                nc.scalar.activation(out_sb[:], ps_fc[:],
                                     mybir.ActivationFunctionType.Identity,
                                     bias=fcb_sb[:])
                nc.sync.dma_start(out_d[:, ts(s, BS)], out_sb[:])

    _strip_redundant_self_waits(nc)
    if RELAX:
        _relax_tanh_waits(nc, mybir, n_fp8)
    return nc


def _relax_tanh_waits(nc, mybir, n_fp8):
    """Point each steady-state Tanh ACT's PE-sem wait a few matmuls EARLIER
    than the last accumulating MM of its PSUM tile.

    The PSUM->ACT handoff costs ~199 ns (SEM_DELAY 100 + seq fetch/decode +
    dispatch) measured from the sem update of the MM the ACT waits on.  The
    trailing DR MMs of the burst only need 13 ns each (27 ns for the bf16
    tail), so waiting on MM #N-delta keeps the data-complete point well
    inside the handoff latency while starting the handoff earlier --
    removing delta MM productions from the serial recurrence cycle.

    Margins (sim-calibrated): fp8 steps delta=6 -> 199-6*13 = 121 ns; bf16
    tail delta=2 -> 199-2*27 = 145 ns.  The 100 ns hardware semaphore
    propagation alone covers the trailing work in both cases (78 ns / 54 ns
    of trailing MM production).  Steps t<2 are skipped (their MMs can run
    at low p-state, 2-4x slower).

    Mechanism: wait VALUES are left untouched (walrus re-derives its
    physical-sem schedule from them; editing a threshold out-of-band
    wedges the device).  Instead the trailing delta MMs' sem-inc updates
    are MOVED onto the (delta+1)-th-from-last MM (update_value 1+delta),
    so the existing threshold is reached delta MMs earlier.  Sem totals
    are unchanged for every later waiter; a pre-pass asserts no OTHER
    instruction waits inside the moved window.
    """
    instrs = [i for b in nc.m.functions[0].blocks for i in b.instructions]

    # Program-order PE Matmults with cumulative per-sem counts, and every
    # wait in the module keyed by sem.
    mms = []                 # (inst, sem, cum_after)
    cum = {}
    all_waits = {}           # sem -> sorted list of (value, inst)
    for i in instrs:
        si = i.sync_info
        if si is None:
            continue
        if type(i).__name__ == "InstMatmult":
            for u in si.on_update:
                assert u.update_mode == "sem-inc" and (u.update_value or 1) == 1
                cum[u.ant_name] = cum.get(u.ant_name, 0) + 1
                mms.append((i, u.ant_name, cum[u.ant_name]))
        for w in si.on_wait:
            if w.wait_mode == "sem-ge-imm":
                all_waits.setdefault(w.ant_name, []).append((w.wait_value, i))

    by_sem_cum = {}          # (sem, cum) -> mm index in mms
    for j, (i, sem, c) in enumerate(mms):
        by_sem_cum[(sem, c)] = j

    t_s = [(t, s) for t in range(T_FULL) for s in range(NS)]
    n_tanh = 0
    for i in instrs:
        if (type(i).__name__ != "InstActivation"
                or i.func != mybir.ActivationFunctionType.Tanh):
            continue
        t, _s = t_s[n_tanh]
        n_tanh += 1
        if t < 2:
            continue
        only = getattr(_relax_tanh_waits, "_only", None)
        if only is not None and (t, _s) not in only:
            continue
        delta = 6 if t < n_fp8 else 2
        si = i.sync_info
        pe_waits = [w for w in (si.on_wait if si else [])
                    if w.wait_mode == "sem-ge-imm" and (w.ant_name, w.wait_value) in by_sem_cum]
        if len(pe_waits) != 1:
            continue
        w = pe_waits[0]
        sem, v = w.ant_name, w.wait_value
        # Nothing else may wait inside (v-delta, v].
        others = [wi for (val, wi) in all_waits.get(sem, ())
                  if v - delta < val <= v and wi is not i]
        if others:
            continue
        j_last = by_sem_cum[(sem, v)]
        j_new = by_sem_cum.get((sem, v - delta))
        if j_new is None or j_last - j_new != delta:
            continue  # window not a contiguous MM run; leave as-is
        # Move the trailing delta increments onto MM j_new.  Attribute
        # mutation on a SyncUpdate does not persist (pyo3 copies), so the
        # on_update LIST is rebuilt with a fresh object.
        import bass_rust
        for j in range(j_new + 1, j_last + 1):
            mi = mms[j][0]
            mi.sync_info.on_update = [u for u in mi.sync_info.on_update
                                      if u.ant_name != sem]
        anchor = mms[j_new][0]
        new_ups = []
        for u in anchor.sync_info.on_update:
            if u.ant_name == sem:
                u = bass_rust.SyncUpdate(
                    sync_type=u.sync_type, id=u.id, update_mode=u.update_mode,
                    ant_name=u.ant_name, update_value=1 + delta,
                    update_reg=u.update_reg)
            new_ups.append(u)
        anchor.sync_info.on_update = new_ups


_SELF_SEM_PREFIX = {
    "InstActivation": "Activation",
    "InstMatmult": "PE",
    "InstLdweights": "PE",
    "InstTensorTensor": "DVE",
    "InstTensorScalarPtr": "DVE",
    "InstTensorCopy": "DVE",
}


def _strip_redundant_self_waits(nc):
    """Drop same-engine semaphore waits from instructions that carry more
    than one sync wait.

    Rationale: the HW engine instruction structs (MM/AC) hold only ONE
    sync-wait command; walrus refuses to codegen instructions with two.
    Tile emits a wait on the instruction's own engine sem for WAW/WAR on
    recycled tile-pool slots, but each engine executes its queue strictly
    in order, so ordering vs. its own earlier instructions is guaranteed
    without the wait.  Cross-engine waits are preserved; sem update counts
    are untouched (no other wait thresholds shift).
    """
    # Semaphore updated by the final DMA store of the "out" tensor; the
    # kernel-tail drain only genuinely needs this one (everything else is
    # transitively ordered: input DMAs -> compute -> final ACT -> out DMA).
    out_dma_sems = set()
    for b in nc.m.functions[0].blocks:
        for i in b.instructions:
            if type(i).__name__ != "InstDMACopy":
                continue
            names = [getattr(ap, "memref", "") for ap in i.outs]
            if "out" in names:
                si = i.sync_info
                if si:
                    out_dma_sems.update(u.ant_name for u in si.on_update)

    for b in nc.m.functions[0].blocks:
        for i in b.instructions:
            si = i.sync_info
            if si is None:
                continue
            ow = si.on_wait
            if len(ow) < 2:
                continue
            tname = type(i).__name__
            if tname == "InstDrain" and any(
                w.ant_name in out_dma_sems for w in ow
            ):
                si.on_wait = [w for w in ow if w.ant_name in out_dma_sems][:1]
                continue
            if tname == "InstDMACopy":
                # Keep the compute-engine wait (real data dependency);
                # drop stale cross-queue DMAHW waits (no data dependency:
                # all earlier DMAs here are input preloads this store
                # does not read, and same-ring descriptors are ordered
                # by the ring itself).
                kept = [w for w in ow if not w.ant_name.startswith("DMA")]
                if kept and len(kept) < len(ow):
                    si.on_wait = kept
                continue
            self_prefix = _SELF_SEM_PREFIX.get(tname)
            if self_prefix is None:
                continue
            kept = [w for w in ow if not w.ant_name.startswith(self_prefix)]
            if kept and len(kept) < len(ow):
                si.on_wait = kept


def _prep_inputs(x, Wx_w, Wx_b, Wh_w, Wh_b, fc_w, fc_b, T):
    """Host-side shard + layout massaging. Returns per-core input maps."""
    bf = ml_dtypes.bfloat16
    f8 = ml_dtypes.float8_e4m3
    bias = (Wx_b + Wh_b).astype(np.float32)

    wxb = WH_SCALE * np.stack([Wx_w.astype(np.float32), bias])       # [2, H]
    whT16 = WH_SCALE * Wh_w.T.astype(np.float32)                     # [Hk, Hm]
    whT = whT16.reshape(KC, 128, HIDDEN).transpose(1, 0, 2).copy().astype(bf)
    wh8 = whT16.reshape(2, 2, 128, HIDDEN).transpose(2, 0, 1, 3).copy().astype(f8)
    fcT = (fc_w.T.astype(np.float32)
           .reshape(KC, 128, HORIZON).transpose(1, 0, 2).copy().astype(bf))
    fcb = fc_b.astype(np.float32).reshape(HORIZON, 1).copy()

    in_maps = []
    for c in range(N_CORES):
        xs = x[c * B_CORE:(c + 1) * B_CORE, :T]                      # [128, T]
        x2T = np.empty((2, T, B_CORE), dtype=np.float32)
        x2T[0] = xs.T
        x2T[1] = 1.0
        boot = np.concatenate(
            [wxb] + [x2T[:, u, :] for u in range(N_BOOT_X)], axis=1)
        x8 = np.concatenate([wxb, x2T.reshape(2, T * B_CORE)], axis=1)
        in_maps.append({
            "x2T": x2T.astype(bf),
            "boot": boot.astype(bf),
            "x8": x8.astype(f8),
            "whT": whT,
            "wh8": wh8,
            "fcT": fcT,
            "fcb": fcb,
        })
    return in_maps


def kernel(x, Wx_w, Wx_b, Wh_w, Wh_b, fc_w, fc_b, _T=T_FULL, _bf16=True,
           _trace=False):
    from concourse.bass_utils import run_bass_kernel_spmd

    key = (_T, _bf16)
    if key not in _COMPILED:
        _COMPILED[key] = build_kernel(T=_T)
    nc = _COMPILED[key]

    in_maps = _prep_inputs(x, Wx_w, Wx_b, Wh_w, Wh_b, fc_w, fc_b, _T)
    try:
        res = run_bass_kernel_spmd(nc, in_maps, list(range(N_CORES)),
                                   trace=_trace)
    except Exception:
        # A rare transient NRT_EXEC_UNIT_UNRECOVERABLE wedge has been seen
        # (~1 in 9 runs); the device recovers on re-execution.
        res = run_bass_kernel_spmd(nc, in_maps, list(range(N_CORES)),
                                   trace=_trace)
    outs = [res.results[c]["out"] for c in range(N_CORES)]               # [24, 128] each
    full = np.concatenate(outs, axis=1).T.astype(np.float32).copy()      # [1024, 24]
    kernel._last_result = res
    return full


# revision 44
# speedup vs baseline: 1.0010x; 1.0007x over previous
"""Trainium2 Bass kernel for nn_CustomRNN_88871463289370.

Reference computation (B=1024, T=256, H=512, HORIZON=24):
    h_0 = 0
    h_{t+1} = tanh(outer(x[:, t], Wx_w) + h_t @ Wh_w.T + (Wx_b + Wh_b))
    out = h_T @ fc_w.T + fc_b                      # [B, 24]

Strategy (data-parallel over batch, 8 cores x 128 rows each):
  * Two independent half-batch STREAMS per core (64 cols each). The
    recurrences are independent, so while stream A sits in its tanh
    latency window (PSUM -> ACT -> SBUF -> sem, ~840 ns), the PE runs
    stream B's matmuls and vice versa. The per-step critical cycle is
    one stream's ACT round-trip (~640 ns) + its 8 recurrent matmuls
    (~105 ns) + the PSUM->ACT hop (~200 ns) = 941 ns/step. More than
    2 streams loses: each extra ACT instruction costs a fixed ~370 ns
    access-latency bubble, making 3+ streams ACT-throughput-bound.
  * Recurrent matmuls in fp8e4m3 with DoubleRow perf mode: one MM
    contracts TWO 128-row k-chunks (weights [128, 2, 128], moving
    h [128, 2, 64]) at 0.5 cycles/row -- 4x less PE time than bf16.
  * Precision: Wh is pre-scaled by 16 before the fp8 cast (a third of
    the uniform(+-1/sqrt(512)) weights sit below e4m3's min normal
    2^-6 unscaled); the tanh activation applies scale=1/16 to PSUM.
    The x-outer phase and its bias stay bf16 (also pre-scaled by 16,
    exact since 16 is a power of two). fp8 h (+-1, ~2% rms quant
    error) leaves a steady-state recurrence error ~1.3e-2, but the
    recurrence is contracting (spectral radius ~0.58), so the LAST
    `TAIL` steps run in bf16 (plain K=128 matmuls), collapsing the
    final error to ~5e-3 (gate: 2e-2).
  * Per stream-step: one [128, 4, 64] PSUM tile (4 hidden m-chunks x
    64 batch cols, 1 KiB -- its own bank), 12 accumulating MMs
    (4 bf16 x-outer K=2 + 8 fp8-DR), one [128, 256] Tanh ACT that
    also casts to the dtype the next step's MMs need (fp8 or bf16).
  * All inputs SBUF-resident after the up-front DMAs; no per-step DMA.
  * Final projection in bf16 from the last step's bf16 h tiles:
    8 K=128 N=64 matmuls into a [24, 128] PSUM tile + Identity ACT
    with per-partition bias.

All host-side reshaping/transposition/casting happens in kernel();
the device kernel sees pre-massaged tensors.
"""

import numpy as np
import ml_dtypes

HIDDEN = 512
HORIZON = 24
B_FULL = 1024
T_FULL = 256
N_CORES = 8
B_CORE = B_FULL // N_CORES  # 128
KC = HIDDEN // 128          # 4 chunks of the hidden dim
NS = 2                      # independent half-batch streams per core
BS = B_CORE // NS           # 64 batch cols per stream
TAIL = 2                    # trailing steps run in bf16 to wash out fp8 error
                            # (numpy-exact sim: rel err 0.0053 vs 2e-2 gate;
                            # HW has tracked the sim within ~1% relative)
N_BOOT_X = 6                # x columns carried in the boot DMA (bf16 ph0);
                            # from t=N_BOOT_X the fp8 x8 tensor has landed
WH_SCALE = 16.0             # power of two: exact to undo via ACT scale

_COMPILED = {}


# _relax_tanh_waits is kept for documentation but DISABLED: both variants
# (reducing ACT wait thresholds out-of-band; moving trailing MM sem-incs
# onto an earlier anchor MM) break downstream layers -- walrus re-derives
# its physical-semaphore schedule from the emitted counts and the edited
# program wedges the device (NRT INTERNAL on both HW attempts, baseline
# healthy in between), and TimelineSim hardcodes sem-inc as +1 so the
# anchor variant deadlocks in sim.  The ~52 ns/step it would save is not
# expressible through Tile's data-dependency model.
RELAX = False


def build_kernel(T=T_FULL, use_bf16=True, tail=TAIL):
    import concourse.bass as bass
    import concourse.mybir as mybir
    import concourse.tile as tile
    from concourse.bass import ts

    dtb = mybir.dt.bfloat16
    dt8 = mybir.dt.float8e4
    f32 = mybir.dt.float32
    DR = mybir.MatmulPerfMode.DoubleRow

    nc = bass.Bass("TRN2", target_bir_lowering=False, debug=False,
                   num_devices=N_CORES)

    # ---- DRAM I/O (per-core shapes; host pre-massages layouts) ----
    # x2T[0, t] = x[:, t], x2T[1, t] = ones; shape [2, T, B_CORE]
    x2T_d = nc.dram_tensor("x2T", [2, T, B_CORE], dtb, kind="ExternalInput").ap()
    # boot[:, :H] = [16*Wx_w ; 16*(Wx_b + Wh_b)], boot[:, H + t*B : H + (t+1)*B]
    # = [x[:, t] ; ones] for t < N_BOOT_X
    boot_d = nc.dram_tensor("boot", [2, HIDDEN + N_BOOT_X * B_CORE], dtb,
                            kind="ExternalInput").ap()
    # fp8 x-outer operands for steps N_BOOT_X..n_fp8-1, one tensor:
    # x8[:, :H] = [16*Wx_w ; 16*(Wx_b+Wh_b)], x8[:, H + t*B + b] = [x[b,t] ; 1]
    # (keeps each fp8 step's PSUM accumulation group single-dtype -- the
    # mixed bf16+fp8 groups are the prime suspect for a rare
    # NRT_EXEC_UNIT_UNRECOVERABLE wedge seen once in ~9 HW runs)
    x8_d = nc.dram_tensor("x8", [2, HIDDEN + T * B_CORE], dt8,
                          kind="ExternalInput").ap()
    # bf16 tail weights [128, KC, H]: whT[p, k, m] = 16*Wh_w[m, k*128+p]
    whT_d = nc.dram_tensor("whT", [128, KC, HIDDEN], dtb, kind="ExternalInput").ap()
    # fp8 DoubleRow weights [128, 2, 2, H]:
    #   wh8[p, pr, i, m] = 16*Wh_w[m, (2*pr+i)*128+p]
    wh8_d = nc.dram_tensor("wh8", [128, 2, 2, HIDDEN], dt8, kind="ExternalInput").ap()
    # fcT arranged [128, KC, HORIZON]: fcT[p, k, n] = fc_w[n, k*128+p] (unscaled)
    fcT_d = nc.dram_tensor("fcT", [128, KC, HORIZON], dtb, kind="ExternalInput").ap()
    # fc_b as column [HORIZON, 1] fp32
    fcb_d = nc.dram_tensor("fcb", [HORIZON, 1], f32, kind="ExternalInput").ap()
    # output [HORIZON, B_CORE] fp32 (host transposes/concats)
    out_d = nc.dram_tensor("out", [HORIZON, B_CORE], f32, kind="ExternalOutput").ap()

    n_fp8 = max(0, T - tail)  # steps 1..n_fp8-1 use fp8-DR recurrent MMs

    with tile.TileContext(nc) as tc:
        with (
            tc.tile_pool(name="consts", bufs=1) as cpool,
            tc.tile_pool(name="h", bufs=3) as hpool,
            tc.tile_pool(name="ps", bufs=3, space="PSUM") as pspool,
            tc.tile_pool(name="psf", bufs=1, space="PSUM") as psfpool,
            tc.tile_pool(name="fin", bufs=1) as finpool,
        ):
            # ---- load constants into SBUF ----
            # Issue order matters: the SP engine serializes dma_start issues
            # (~650 ns each). t=0 needs wxb+x2, t=1 needs wh8; whT is only
            # read from step T-TAIL (~230 us in), so it goes last.
            # "boot" tile: wxb + the first N_BOOT_X x columns in ONE first
            # DMA -- the serial per-DMA issue+descgen+completion latency
            # (~2 us each) is what gates the first matmuls, so everything
            # steps 0..N_BOOT_X-1 need (except wh8) rides together.  wh8
            # goes second (it gates step 1's recurrent MMs); the bulk x
            # transfer third (first read at t=N_BOOT_X, ~1.5 us later).
            boot_sb = cpool.tile([2, HIDDEN + N_BOOT_X * B_CORE], dtb)
            nc.sync.dma_start(boot_sb[:], boot_d[:])
            wh8_sb = cpool.tile([128, 2, 2, HIDDEN], dt8)
            nc.sync.dma_start(wh8_sb[:], wh8_d[:])
            x8_sb = cpool.tile([2, HIDDEN + T * B_CORE], dt8)
            nc.sync.dma_start(x8_sb[:], x8_d[:])
            fcT_sb = cpool.tile([128, KC, HORIZON], dtb)
            nc.sync.dma_start(fcT_sb[:], fcT_d[:])
            fcb_sb = cpool.tile([HORIZON, 1], f32)
            nc.sync.dma_start(fcb_sb[:], fcb_d[:])
            # x2 (bf16) is only read by the bf16 TAIL steps (~230 us in);
            # whT likewise -- both ride at the back of the queue.
            x2_sb = cpool.tile([2, T, B_CORE], dtb)
            nc.sync.dma_start(x2_sb[:], x2T_d[:])
            whT_sb = cpool.tile([128, KC, HIDDEN], dtb)
            nc.sync.dma_start(whT_sb[:], whT_d[:])
            # Touch fcb on ScalarE right away so the DMA wait lands here,
            # not on the final bias activation (which already carries a PE
            # wait; the AC instruction struct fits only one sync wait).
            fcb_scratch = cpool.tile([1, 1], f32)
            nc.scalar.activation(fcb_scratch[:], fcb_sb[0:1, 0:1],
                                 mybir.ActivationFunctionType.Identity)

            inv = 1.0 / WH_SCALE
            h = [None, None]  # per-stream [128, KC, BS] tiles

            def hk(hs, k):
                """Chunk k of a previous-step h: single tile or (h01, h23)."""
                if isinstance(hs, tuple):
                    return hs[k // 2][:, k % 2, :]
                return hs[:, k, :]

            for t in range(T):
                fp8_mm = 0 < t < n_fp8       # this step's recurrent MM flavor
                fp8_out = (t + 1) < n_fp8    # dtype the NEXT step's MMs need
                if t >= n_fp8:
                    # bf16 TAIL step, k-phase split: two half-PSUMs + two
                    # half-ACTs per stream so the m01 tanh (and with it the
                    # next step's k01 matmuls / the fc projection) starts
                    # ~200 ns earlier.  Summation order per element is
                    # unchanged (ph0 then k=0..3), so results are
                    # bit-identical to the unsplit path.
                    for s in range(NS):
                        halves = []
                        for hh in range(2):          # hh=0: m0,m1; 1: m2,m3
                            psh = pspool.tile([128, 2, BS], f32, tag=f"ps{hh}")
                            xsrc = x2_sb[0:2, t, ts(s, BS)]
                            for mm in range(2):
                                m = 2 * hh + mm
                                nc.tensor.matmul(psh[:, mm, :],
                                                 boot_sb[0:2, ts(m, 128)],
                                                 xsrc,
                                                 start=(mm == 0), stop=False)
                            for mm in range(2):
                                m = 2 * hh + mm
                                for k in range(KC):
                                    nc.tensor.matmul(
                                        psh[:, mm, :],
                                        whT_sb[:, k, ts(m, 128)],
                                        hk(h[s], k),
                                        start=False,
                                        stop=(mm == 1 and k == KC - 1))
                            h_half = hpool.tile([128, 2, BS], dtb,
                                                tag=f"hb{s}")
                            nc.scalar.activation(
                                h_half[:], psh[:],
                                mybir.ActivationFunctionType.Tanh, scale=inv)
                            halves.append(h_half)
                        h[s] = (halves[0], halves[1])
                    continue
                for s in range(NS):
                    ps = pspool.tile([128, KC, BS], f32, tag=f"ps{s}")
                    # x-outer + bias (K=2). fp8 during the fp8 steps so the
                    # whole accumulation group is one dtype; bf16 (boot/x2)
                    # for t<N_BOOT_X and the tail. One start per PSUM bank,
                    # one stop on the bank's last accumulating MM.
                    if N_BOOT_X <= t < n_fp8:
                        off = HIDDEN + t * B_CORE + s * BS
                        wsrc, xsrc = x8_sb, x8_sb[0:2, off:off + BS]
                    else:
                        wsrc = boot_sb
                        xsrc = (boot_sb[0:2, HIDDEN + t * B_CORE + s * BS:
                                        HIDDEN + t * B_CORE + (s + 1) * BS]
                                if t < N_BOOT_X else x2_sb[0:2, t, ts(s, BS)])
                    for m in range(KC):
                        nc.tensor.matmul(ps[:, m, :],
                                         wsrc[0:2, ts(m, 128)],
                                         xsrc,
                                         start=(m == 0),
                                         stop=(t == 0 and m == KC - 1))
                    if t > 0:
                        if fp8_mm:
                            # fp8 DoubleRow: contract k-chunk pair (2pr, 2pr+1)
                            for m in range(KC):
                                for pr in range(2):
                                    nc.tensor.matmul(
                                        ps[:, m, :],
                                        wh8_sb[:, pr, :, ts(m, 128)],
                                        h[s][:, 2 * pr:2 * pr + 2, :],
                                        start=False,
                                        stop=(m == KC - 1 and pr == 1),
                                        perf_mode=DR)
                        else:
                            for m in range(KC):
                                for k in range(KC):
                                    nc.tensor.matmul(
                                        ps[:, m, :],
                                        whT_sb[:, k, ts(m, 128)],
                                        h[s][:, k, :],
                                        start=False,
                                        stop=(m == KC - 1 and k == KC - 1))
                    htag = f"h8{s}" if fp8_out else f"hb{s}"
                    h_new = hpool.tile([128, KC, BS], dt8 if fp8_out else dtb,
                                       tag=htag)
                    nc.scalar.activation(h_new[:], ps[:],
                                         mybir.ActivationFunctionType.Tanh,
                                         scale=inv)
                    h[s] = h_new

            # ---- final projection: out[n, b] = sum_k fcT[k].T @ h[k] + b ----
            # Per-stream matmuls + bias-ACTs (A's overlap B's last tanh),
            # then ONE store: two DMAs would serialize on the SP issue
            # queue (~650 ns apart), landing later than the single DMA
            # gated on stream B's bias-ACT.
            out_sb = finpool.tile([HORIZON, B_CORE], f32)
            for s in range(NS):
                ps_fc = psfpool.tile([HORIZON, BS], f32, tag=f"psfc{s}")
                for k in range(KC):
                    nc.tensor.matmul(ps_fc[:],
                                     fcT_sb[:, k, :],
                                     hk(h[s], k),
                                     start=(k == 0),
                                     stop=(k == KC - 1))
                nc.scalar.activation(out_sb[:, ts(s, BS)], ps_fc[:],
                                     mybir.ActivationFunctionType.Identity,
                                     bias=fcb_sb[:])
            nc.sync.dma_start(out_d[:], out_sb[:])

    _strip_redundant_self_waits(nc)
    if RELAX:
        _relax_tanh_waits(nc, mybir, n_fp8)
    return nc


def _relax_tanh_waits(nc, mybir, n_fp8):
    """Point each steady-state Tanh ACT's PE-sem wait a few matmuls EARLIER
    than the last accumulating MM of its PSUM tile.

    The PSUM->ACT handoff costs ~199 ns (SEM_DELAY 100 + seq fetch/decode +
    dispatch) measured from the sem update of the MM the ACT waits on.  The
    trailing DR MMs of the burst only need 13 ns each (27 ns for the bf16
    tail), so waiting on MM #N-delta keeps the data-complete point well
    inside the handoff latency while starting the handoff earlier --
    removing delta MM productions from the serial recurrence cycle.

    Margins (sim-calibrated): fp8 steps delta=6 -> 199-6*13 = 121 ns; bf16
    tail delta=2 -> 199-2*27 = 145 ns.  The 100 ns hardware semaphore
    propagation alone covers the trailing work in both cases (78 ns / 54 ns
    of trailing MM production).  Steps t<2 are skipped (their MMs can run
    at low p-state, 2-4x slower).

    Mechanism: wait VALUES are left untouched (walrus re-derives its
    physical-sem schedule from them; editing a threshold out-of-band
    wedges the device).  Instead the trailing delta MMs' sem-inc updates
    are MOVED onto the (delta+1)-th-from-last MM (update_value 1+delta),
    so the existing threshold is reached delta MMs earlier.  Sem totals
    are unchanged for every later waiter; a pre-pass asserts no OTHER
    instruction waits inside the moved window.
    """
    instrs = [i for b in nc.m.functions[0].blocks for i in b.instructions]

    # Program-order PE Matmults with cumulative per-sem counts, and every
    # wait in the module keyed by sem.
    mms = []                 # (inst, sem, cum_after)
    cum = {}
    all_waits = {}           # sem -> sorted list of (value, inst)
    for i in instrs:
        si = i.sync_info
        if si is None:
            continue
        if type(i).__name__ == "InstMatmult":
            for u in si.on_update:
                assert u.update_mode == "sem-inc" and (u.update_value or 1) == 1
                cum[u.ant_name] = cum.get(u.ant_name, 0) + 1
                mms.append((i, u.ant_name, cum[u.ant_name]))
        for w in si.on_wait:
            if w.wait_mode == "sem-ge-imm":
                all_waits.setdefault(w.ant_name, []).append((w.wait_value, i))

    by_sem_cum = {}          # (sem, cum) -> mm index in mms
    for j, (i, sem, c) in enumerate(mms):
        by_sem_cum[(sem, c)] = j

    t_s = [(t, s) for t in range(T_FULL) for s in range(NS)]
    n_tanh = 0
    for i in instrs:
        if (type(i).__name__ != "InstActivation"
                or i.func != mybir.ActivationFunctionType.Tanh):
            continue
        t, _s = t_s[n_tanh]
        n_tanh += 1
        if t < 2:
            continue
        only = getattr(_relax_tanh_waits, "_only", None)
        if only is not None and (t, _s) not in only:
            continue
        delta = 6 if t < n_fp8 else 2
        si = i.sync_info
        pe_waits = [w for w in (si.on_wait if si else [])
                    if w.wait_mode == "sem-ge-imm" and (w.ant_name, w.wait_value) in by_sem_cum]
        if len(pe_waits) != 1:
            continue
        w = pe_waits[0]
        sem, v = w.ant_name, w.wait_value
        # Nothing else may wait inside (v-delta, v].
        others = [wi for (val, wi) in all_waits.get(sem, ())
                  if v - delta < val <= v and wi is not i]
        if others:
            continue
        j_last = by_sem_cum[(sem, v)]
        j_new = by_sem_cum.get((sem, v - delta))
        if j_new is None or j_last - j_new != delta:
            continue  # window not a contiguous MM run; leave as-is
        # Move the trailing delta increments onto MM j_new.  Attribute
        # mutation on a SyncUpdate does not persist (pyo3 copies), so the
        # on_update LIST is rebuilt with a fresh object.
        import bass_rust
        for j in range(j_new + 1, j_last + 1):
            mi = mms[j][0]
            mi.sync_info.on_update = [u for u in mi.sync_info.on_update
                                      if u.ant_name != sem]
        anchor = mms[j_new][0]
        new_ups = []
        for u in anchor.sync_info.on_update:
            if u.ant_name == sem:
                u = bass_rust.SyncUpdate(
                    sync_type=u.sync_type, id=u.id, update_mode=u.update_mode,
                    ant_name=u.ant_name, update_value=1 + delta,
                    update_reg=u.update_reg)
            new_ups.append(u)
        anchor.sync_info.on_update = new_ups


_SELF_SEM_PREFIX = {
    "InstActivation": "Activation",
    "InstMatmult": "PE",
    "InstLdweights": "PE",
    "InstTensorTensor": "DVE",
    "InstTensorScalarPtr": "DVE",
    "InstTensorCopy": "DVE",
}


def _strip_redundant_self_waits(nc):
    """Drop same-engine semaphore waits from instructions that carry more
    than one sync wait.

    Rationale: the HW engine instruction structs (MM/AC) hold only ONE
    sync-wait command; walrus refuses to codegen instructions with two.
    Tile emits a wait on the instruction's own engine sem for WAW/WAR on
    recycled tile-pool slots, but each engine executes its queue strictly
    in order, so ordering vs. its own earlier instructions is guaranteed
    without the wait.  Cross-engine waits are preserved; sem update counts
    are untouched (no other wait thresholds shift).
    """
    # Semaphore updated by the final DMA store of the "out" tensor; the
    # kernel-tail drain only genuinely needs this one (everything else is
    # transitively ordered: input DMAs -> compute -> final ACT -> out DMA).
    out_dma_sems = set()
    for b in nc.m.functions[0].blocks:
        for i in b.instructions:
            if type(i).__name__ != "InstDMACopy":
                continue
            names = [getattr(ap, "memref", "") for ap in i.outs]
            if "out" in names:
                si = i.sync_info
                if si:
                    out_dma_sems.update(u.ant_name for u in si.on_update)

    for b in nc.m.functions[0].blocks:
        for i in b.instructions:
            si = i.sync_info
            if si is None:
                continue
            ow = si.on_wait
            if len(ow) < 2:
                continue
            tname = type(i).__name__
            if tname == "InstDrain" and any(
                w.ant_name in out_dma_sems for w in ow
            ):
                si.on_wait = [w for w in ow if w.ant_name in out_dma_sems][:1]
                continue
            if tname == "InstDMACopy":
                # Keep the compute-engine wait (real data dependency);
                # drop stale cross-queue DMAHW waits (no data dependency:
                # all earlier DMAs here are input preloads this store
                # does not read, and same-ring descriptors are ordered
                # by the ring itself).
                kept = [w for w in ow if not w.ant_name.startswith("DMA")]
                if kept and len(kept) < len(ow):
                    si.on_wait = kept
                continue
            self_prefix = _SELF_SEM_PREFIX.get(tname)
            if self_prefix is None:
                continue
            kept = [w for w in ow if not w.ant_name.startswith(self_prefix)]
            if kept and len(kept) < len(ow):
                si.on_wait = kept


def _prep_inputs(x, Wx_w, Wx_b, Wh_w, Wh_b, fc_w, fc_b, T):
    """Host-side shard + layout massaging. Returns per-core input maps."""
    bf = ml_dtypes.bfloat16
    f8 = ml_dtypes.float8_e4m3
    bias = (Wx_b + Wh_b).astype(np.float32)

    wxb = WH_SCALE * np.stack([Wx_w.astype(np.float32), bias])       # [2, H]
    whT16 = WH_SCALE * Wh_w.T.astype(np.float32)                     # [Hk, Hm]
    whT = whT16.reshape(KC, 128, HIDDEN).transpose(1, 0, 2).copy().astype(bf)
    wh8 = whT16.reshape(2, 2, 128, HIDDEN).transpose(2, 0, 1, 3).copy().astype(f8)
    fcT = (fc_w.T.astype(np.float32)
           .reshape(KC, 128, HORIZON).transpose(1, 0, 2).copy().astype(bf))
    fcb = fc_b.astype(np.float32).reshape(HORIZON, 1).copy()

    in_maps = []
    for c in range(N_CORES):
        xs = x[c * B_CORE:(c + 1) * B_CORE, :T]                      # [128, T]
        x2T = np.empty((2, T, B_CORE), dtype=np.float32)
        x2T[0] = xs.T
        x2T[1] = 1.0
        boot = np.concatenate(
            [wxb] + [x2T[:, u, :] for u in range(N_BOOT_X)], axis=1)
        x8 = np.concatenate([wxb, x2T.reshape(2, T * B_CORE)], axis=1)
        in_maps.append({
            "x2T": x2T.astype(bf),
            "boot": boot.astype(bf),
            "x8": x8.astype(f8),
            "whT": whT,
            "wh8": wh8,
            "fcT": fcT,
            "fcb": fcb,
        })
    return in_maps


def kernel(x, Wx_w, Wx_b, Wh_w, Wh_b, fc_w, fc_b, _T=T_FULL, _bf16=True,
           _trace=False):
    from concourse.bass_utils import run_bass_kernel_spmd

    key = (_T, _bf16)
    if key not in _COMPILED:
        _COMPILED[key] = build_kernel(T=_T)
    nc = _COMPILED[key]

    in_maps = _prep_inputs(x, Wx_w, Wx_b, Wh_w, Wh_b, fc_w, fc_b, _T)
    try:
        res = run_bass_kernel_spmd(nc, in_maps, list(range(N_CORES)),
                                   trace=_trace)
    except Exception:
        # A rare transient NRT_EXEC_UNIT_UNRECOVERABLE wedge has been seen
        # (~1 in 9 runs); the device recovers on re-execution.
        res = run_bass_kernel_spmd(nc, in_maps, list(range(N_CORES)),
                                   trace=_trace)
    outs = [res.results[c]["out"] for c in range(N_CORES)]               # [24, 128] each
    full = np.concatenate(outs, axis=1).T.astype(np.float32).copy()      # [1024, 24]
    kernel._last_result = res
    return full
